# revision 1
# baseline (speedup 1.0000x reference)
"""Trainium2 Bass kernel for nn_ItemVectorTransform.

reference:
    scores = exp(x @ memory.T)        # [B, K]
    u_read = scores @ memory          # [B, D]
    out    = concat([x, u_read], -1)  # [B, 2D]

B=65536, K=2048, D=50. Data-parallel over 8 NeuronCores (8192 rows each),
memory table replicated.

Per-core dataflow (all compute on-chip, scores never touch HBM):
  - memory loaded once; PE-transposed to memT [D, K] (f32r) for mm1;
    cast to bf16 [K, D] chunks for mm2.
  - loop over 4 batch macro-tiles of 2048 rows:
      x tile load -> PE transpose -> xT [D, 2048] (f32r)
      mm1 (f32r): scoresT chunk [128k, 1024b] in PSUM
      exp on ACT: PSUM -> SBUF bf16 scores
      mm2 (bf16): u[128b, D] accumulated over 16 k-chunks in PSUM
      assemble [128, 100] out tile (x passthrough + u) -> DMA out
"""

import sys

sys.path.insert(0, "/opt/trn_rl_repo")

import numpy as np

B, K, D = 65536, 2048, 50
N_CORES = 8
B_CORE = B // N_CORES  # 8192

B_MACRO = 2048          # batch rows per macro tile
N_MACRO = B_CORE // B_MACRO
KC = K // 128           # 16 k-chunks
SM = B_MACRO // 128     # 16 x sub-tiles per macro
S_W = 1024              # exp / psum_s width
N_H = B_MACRO // S_W

_built = None


def _build():
    import concourse.tile as tile
    from concourse import bacc, mybir
    from concourse.masks import make_identity

    f32 = mybir.dt.float32
    f32r = mybir.dt.float32r
    bf16 = mybir.dt.bfloat16
    Exp = mybir.ActivationFunctionType.Exp

    nc = bacc.Bacc("TRN2", target_bir_lowering=False, debug=False)
    x_d = nc.dram_tensor("x", [B_CORE, D], f32, kind="ExternalInput").ap()
    m_d = nc.dram_tensor("memory", [K, D], f32, kind="ExternalInput").ap()
    o_d = nc.dram_tensor("out", [B_CORE, 2 * D], f32, kind="ExternalOutput").ap()

    with tile.TileContext(nc) as tc:
        with (
            tc.tile_pool(name="singles", bufs=1) as singles,
            tc.tile_pool(name="xmac", bufs=2) as xmac,
            tc.tile_pool(name="sexp", bufs=2) as sexp_pool,
            tc.tile_pool(name="outp", bufs=4) as outp,
            tc.tile_pool(name="pt", bufs=2, space="PSUM") as pt_pool,
            tc.tile_pool(name="ps", bufs=2, space="PSUM") as ps_pool,
            tc.tile_pool(name="pu", bufs=2, space="PSUM") as pu_pool,
        ):
            ident = singles.tile([128, 128], f32)
            make_identity(nc, ident[:])

            # memory natural layout [128, KC, D]: [p, s, d] = memory[s*128+p, d]
            mem_nat = singles.tile([128, KC, D], f32)
            nc.sync.dma_start(
                out=mem_nat[:], in_=m_d.rearrange("(s p) d -> p s d", p=128)
            )
            mem_bf = singles.tile([128, KC, D], bf16)
            memT = singles.tile([D, K], f32r)
            for s in range(KC):
                nc.vector.tensor_copy(mem_bf[:, s, :], mem_nat[:, s, :])
                p_t = pt_pool.tile([D, 128], f32, tag="pt")
                nc.tensor.transpose(p_t[:], mem_nat[:, s, :], ident[:])
                nc.vector.tensor_copy(memT[:, s * 128 : (s + 1) * 128], p_t[:])

            for mi in range(N_MACRO):
                b0 = mi * B_MACRO
                x_nat = xmac.tile([128, SM, D], f32, tag="x_nat")
                nc.sync.dma_start(
                    out=x_nat[:],
                    in_=x_d[b0 : b0 + B_MACRO, :].rearrange(
                        "(s p) d -> p s d", p=128
                    ),
                )
                xT = xmac.tile([D, B_MACRO], f32r, tag="xT")
                for s in range(SM):
                    p_t = pt_pool.tile([D, 128], f32, tag="pt")
                    nc.tensor.transpose(p_t[:], x_nat[:, s, :], ident[:])
                    nc.vector.tensor_copy(xT[:, s * 128 : (s + 1) * 128], p_t[:])

                s_exp = sexp_pool.tile([128, KC, B_MACRO], bf16, tag="s_exp")
                for k in range(KC):
                    lhsT = memT[:, k * 128 : (k + 1) * 128]
                    for h in range(N_H):
                        p_s = ps_pool.tile([128, S_W], f32, tag="ps")
                        for j in range(S_W // 512):
                            off = h * S_W + j * 512
                            nc.tensor.matmul(
                                p_s[:, j * 512 : (j + 1) * 512],
                                lhsT,
                                xT[:, off : off + 512],
                                start=True,
                                stop=True,
                            )
                        nc.scalar.activation(
                            s_exp[:, k, h * S_W : (h + 1) * S_W], p_s[:], Exp
                        )

                for s in range(SM):
                    p_u = pu_pool.tile([128, D], f32, tag="pu")
                    for k in range(KC):
                        nc.tensor.matmul(
                            p_u[:],
                            s_exp[:, k, s * 128 : (s + 1) * 128],
                            mem_bf[:, k, :],
                            start=(k == 0),
                            stop=(k == KC - 1),
                        )
                    o_t = outp.tile([128, 2 * D], f32, tag="o_t")
                    nc.vector.tensor_copy(o_t[:, :D], x_nat[:, s, :])
                    nc.vector.tensor_copy(o_t[:, D:], p_u[:])
                    nc.sync.dma_start(
                        out=o_d[b0 + s * 128 : b0 + (s + 1) * 128, :], in_=o_t[:]
                    )

    nc.compile()
    return nc


def _get_nc():
    global _built
    if _built is None:
        _built = _build()
    return _built


def run_spmd(x, memory, **spmd_kwargs):
    """Run the kernel; returns (full_output, BassKernelResults)."""
    from concourse.bass_utils import run_bass_kernel_spmd

    nc = _get_nc()
    x = np.ascontiguousarray(x, dtype=np.float32)
    memory = np.ascontiguousarray(memory, dtype=np.float32)
    in_maps = [
        {
            "x": np.ascontiguousarray(x[i * B_CORE : (i + 1) * B_CORE]),
            "memory": memory,
        }
        for i in range(N_CORES)
    ]
    res = run_bass_kernel_spmd(nc, in_maps, core_ids=list(range(N_CORES)), **spmd_kwargs)
    out = np.concatenate([res.results[i]["out"] for i in range(N_CORES)], axis=0)
    return out, res


def kernel(x, memory):
    out, _ = run_spmd(x, memory)
    return out


# revision 8
# speedup vs baseline: 1.8245x; 1.8245x over previous
"""Trainium2 Bass kernel for nn_ItemVectorTransform.

reference:
    scores = exp(x @ memory.T)        # [B, K]
    u_read = scores @ memory          # [B, D]
    out    = concat([x, u_read], -1)  # [B, 2D]

B=65536, K=2048, D=50. Data-parallel over 8 NeuronCores (8192 rows each),
memory table replicated.

Per-core dataflow (all compute on-chip, scores never touch HBM):
  - memory loaded once; PE-transposed to memT [D, K] (f32r) for mm1;
    cast to bf16 [K, D] chunks for mm2.
  - loop over 4 batch macro-tiles of 2048 rows:
      x tile load -> PE transpose -> xT [D, 2048] (f32r)
      mm1 (f32r): scoresT chunk [128k, 1024b] in PSUM
      exp on ACT: PSUM -> SBUF bf16 scores
      mm2 (bf16): u[128b, D] accumulated over 16 k-chunks in PSUM
      assemble [128, 100] out tile (x passthrough + u) -> DMA out
"""

import sys

sys.path.insert(0, "/opt/trn_rl_repo")

import numpy as np

B, K, D = 65536, 2048, 50
N_CORES = 8
B_CORE = B // N_CORES  # 8192

B_MACRO = 2048          # batch rows per macro tile
N_MACRO = B_CORE // B_MACRO
KC = K // 128           # 16 k-chunks
SM = B_MACRO // 128     # 16 x sub-tiles per macro
S_W = 2048              # exp / psum_s width
N_H = B_MACRO // S_W

_built = None
REPS = 1  # bench-only: replicate the whole computation inside one NEFF


def _build():
    import concourse.tile as tile
    from concourse import bacc, mybir
    from concourse.masks import make_identity

    f32 = mybir.dt.float32
    f32r = mybir.dt.float32r
    bf16 = mybir.dt.bfloat16
    Exp = mybir.ActivationFunctionType.Exp

    nc = bacc.Bacc("TRN2", target_bir_lowering=False, debug=False)
    x_d = nc.dram_tensor("x", [B_CORE, D], f32, kind="ExternalInput").ap()
    m_d = nc.dram_tensor("memory", [K, D], f32, kind="ExternalInput").ap()
    o_d = nc.dram_tensor("out", [B_CORE, 2 * D], f32, kind="ExternalOutput").ap()

    with tile.TileContext(nc) as tc:
        with (
            tc.tile_pool(name="singles", bufs=1) as singles,
            tc.tile_pool(name="xmac", bufs=2) as xmac,
            tc.tile_pool(name="sexp", bufs=2) as sexp_pool,
            tc.tile_pool(name="outp", bufs=4) as outp,
            tc.tile_pool(name="ps", bufs=2, space="PSUM") as ps_pool,
        ):
            # One shared 2-slot [128, S_W] PSUM pool (8 banks total): mm1
            # outputs, PE-transpose outputs, and mm2 accumulators time-share
            # the same slots (tag "ps"), so exp can read a full 2048-wide
            # PSUM tile in a single ACT instruction.
            pt_pool = ps_pool
            pu_pool = ps_pool
            ident = singles.tile([128, 128], f32)
            make_identity(nc, ident[:])

            # memory natural layout [128, KC, D]: [p, s, d] = memory[s*128+p, d]
            mem_nat = singles.tile([128, KC, D], f32)
            nc.sync.dma_start(
                out=mem_nat[:], in_=m_d.rearrange("(s p) d -> p s d", p=128)
            )
            mem_bf = singles.tile([128, KC, D], bf16)
            memT = singles.tile([D, K], f32r)
            for s in range(KC):
                nc.vector.tensor_copy(mem_bf[:, s, :], mem_nat[:, s, :])
                p_t = pt_pool.tile([D, 128], f32, tag="ps")
                nc.tensor.transpose(p_t[:], mem_nat[:, s, :], ident[:])
                nc.vector.tensor_copy(memT[:, s * 128 : (s + 1) * 128], p_t[:])

            # Software pipeline over macros: phase A (x load/transpose, mm1+exp)
            # of macro mi is emitted interleaved with phase B (mm2, output) of
            # macro mi-1, so the in-order PE always has mm2 work to run while
            # ACT (the bottleneck) drains the exp queue.
            n_mac = N_MACRO * REPS
            prev = None  # (x_nat, s_exp, b0) of macro mi-1
            for mi in range(n_mac + 1):
                cur = None
                if mi < n_mac:
                    b0 = (mi % N_MACRO) * B_MACRO
                    x_nat = xmac.tile([128, SM, D], f32, tag="x_nat")
                    nc.sync.dma_start(
                        out=x_nat[:],
                        in_=x_d[b0 : b0 + B_MACRO, :].rearrange(
                            "(s p) d -> p s d", p=128
                        ),
                    )
                    xT = xmac.tile([D, B_MACRO], f32r, tag="xT")
                    for s in range(SM):
                        p_t = pt_pool.tile([D, 128], f32, tag="ps")
                        nc.tensor.transpose(p_t[:], x_nat[:, s, :], ident[:])
                        nc.vector.tensor_copy(xT[:, s * 128 : (s + 1) * 128], p_t[:])
                    s_exp = sexp_pool.tile([128, KC, B_MACRO], bf16, tag="s_exp")
                    cur = (x_nat, s_exp, b0)

                for k in range(KC):
                    if mi < n_mac:
                        lhsT = memT[:, k * 128 : (k + 1) * 128]
                        for h in range(N_H):
                            p_s = ps_pool.tile([128, S_W], f32, tag="ps")
                            for j in range(S_W // 512):
                                off = h * S_W + j * 512
                                nc.tensor.matmul(
                                    p_s[:, j * 512 : (j + 1) * 512],
                                    lhsT,
                                    xT[:, off : off + 512],
                                    start=True,
                                    stop=True,
                                )
                            nc.scalar.activation(
                                s_exp[:, k, h * S_W : (h + 1) * S_W], p_s[:], Exp
                            )
                    if prev is not None:
                        px_nat, ps_exp, pb0 = prev
                        s = k  # one mm2 output group per k-slot
                        p_u = pu_pool.tile([128, D], f32, tag="ps")
                        for kk in range(KC):
                            nc.tensor.matmul(
                                p_u[:],
                                ps_exp[:, kk, s * 128 : (s + 1) * 128],
                                mem_bf[:, kk, :],
                                start=(kk == 0),
                                stop=(kk == KC - 1),
                            )
                        o_t = outp.tile([128, 2 * D], f32, tag="o_t")
                        nc.vector.tensor_copy(o_t[:, :D], px_nat[:, s, :])
                        nc.vector.tensor_copy(o_t[:, D:], p_u[:])
                        nc.sync.dma_start(
                            out=o_d[pb0 + s * 128 : pb0 + (s + 1) * 128, :],
                            in_=o_t[:],
                        )
                prev = cur

    nc.compile()
    return nc


def _get_nc():
    global _built
    if _built is None:
        _built = _build()
    return _built


def run_spmd(x, memory, **spmd_kwargs):
    """Run the kernel; returns (full_output, BassKernelResults)."""
    from concourse.bass_utils import run_bass_kernel_spmd

    nc = _get_nc()
    x = np.ascontiguousarray(x, dtype=np.float32)
    memory = np.ascontiguousarray(memory, dtype=np.float32)
    in_maps = [
        {
            "x": np.ascontiguousarray(x[i * B_CORE : (i + 1) * B_CORE]),
            "memory": memory,
        }
        for i in range(N_CORES)
    ]
    res = run_bass_kernel_spmd(nc, in_maps, core_ids=list(range(N_CORES)), **spmd_kwargs)
    out = np.concatenate([res.results[i]["out"] for i in range(N_CORES)], axis=0)
    return out, res


def kernel(x, memory):
    out, _ = run_spmd(x, memory)
    return out


# revision 9
# speedup vs baseline: 1.9892x; 1.0903x over previous
"""Trainium2 Bass kernel for nn_ItemVectorTransform.

reference:
    scores = exp(x @ memory.T)        # [B, K]
    u_read = scores @ memory          # [B, D]
    out    = concat([x, u_read], -1)  # [B, 2D]

B=65536, K=2048, D=50. Data-parallel over 8 NeuronCores (8192 rows each),
memory table replicated.

Per-core dataflow (all compute on-chip, scores never touch HBM):
  - memory loaded once; PE-transposed to memT [D, K] (f32r) for mm1;
    cast to bf16 [K, D] chunks for mm2.
  - loop over 4 batch macro-tiles of 2048 rows:
      x tile load -> PE transpose -> xT [D, 2048] (f32r)
      mm1 (f32r): scoresT chunk [128k, 1024b] in PSUM
      exp on ACT: PSUM -> SBUF bf16 scores
      mm2 (bf16): u[128b, D] accumulated over 16 k-chunks in PSUM
      assemble [128, 100] out tile (x passthrough + u) -> DMA out
"""

import sys

sys.path.insert(0, "/opt/trn_rl_repo")

import numpy as np

B, K, D = 65536, 2048, 50
N_CORES = 8
B_CORE = B // N_CORES  # 8192

B_MACRO = 2048          # batch rows per macro tile
N_MACRO = B_CORE // B_MACRO
KC = K // 128           # 16 k-chunks
SM = B_MACRO // 128     # 16 x sub-tiles per macro
S_W = 1024              # exp / psum_s width
N_H = B_MACRO // S_W

_built = None
REPS = 1  # bench-only: replicate the whole computation inside one NEFF


def _build():
    import concourse.tile as tile
    from concourse import bacc, mybir
    from concourse.masks import make_identity

    f32 = mybir.dt.float32
    f32r = mybir.dt.float32r
    bf16 = mybir.dt.bfloat16
    Exp = mybir.ActivationFunctionType.Exp

    nc = bacc.Bacc("TRN2", target_bir_lowering=False, debug=False)
    x_d = nc.dram_tensor("x", [B_CORE, D], f32, kind="ExternalInput").ap()
    m_d = nc.dram_tensor("memory", [K, D], f32, kind="ExternalInput").ap()
    o_d = nc.dram_tensor("out", [B_CORE, 2 * D], f32, kind="ExternalOutput").ap()

    with tile.TileContext(nc) as tc:
        with (
            tc.tile_pool(name="singles", bufs=1) as singles,
            tc.tile_pool(name="xmac", bufs=2) as xmac,
            tc.tile_pool(name="sexp", bufs=2) as sexp_pool,
            tc.tile_pool(name="outp", bufs=4) as outp,
            tc.tile_pool(name="ps", bufs=2, space="PSUM") as ps_pool,
            tc.tile_pool(name="sm", bufs=4, space="PSUM") as sm_pool,
        ):
            pt_pool = sm_pool
            pu_pool = sm_pool
            ident = singles.tile([128, 128], f32)
            make_identity(nc, ident[:])

            # memory natural layout [128, KC, D]: [p, s, d] = memory[s*128+p, d]
            mem_nat = singles.tile([128, KC, D], f32)
            nc.sync.dma_start(
                out=mem_nat[:], in_=m_d.rearrange("(s p) d -> p s d", p=128)
            )
            mem_bf = singles.tile([128, KC, D], bf16)
            memT = singles.tile([D, K], f32r)
            for s in range(KC):
                nc.vector.tensor_copy(mem_bf[:, s, :], mem_nat[:, s, :])
                p_t = pt_pool.tile([D, 128], f32, tag="sm")
                nc.tensor.transpose(p_t[:], mem_nat[:, s, :], ident[:])
                nc.vector.tensor_copy(memT[:, s * 128 : (s + 1) * 128], p_t[:])

            # Software pipeline over macros: phase A (x load/transpose, mm1+exp)
            # of macro mi is emitted interleaved with phase B (mm2, output) of
            # macro mi-1, so the in-order PE always has mm2 work to run while
            # ACT (the bottleneck) drains the exp queue.
            n_mac = N_MACRO * REPS
            prev = None  # (x_nat, s_exp, b0) of macro mi-1
            for mi in range(n_mac + 1):
                cur = None
                if mi < n_mac:
                    b0 = (mi % N_MACRO) * B_MACRO
                    x_nat = xmac.tile([128, SM, D], f32, tag="x_nat")
                    nc.sync.dma_start(
                        out=x_nat[:],
                        in_=x_d[b0 : b0 + B_MACRO, :].rearrange(
                            "(s p) d -> p s d", p=128
                        ),
                    )
                    xT = xmac.tile([D, B_MACRO], f32r, tag="xT")
                    for s in range(SM):
                        p_t = pt_pool.tile([D, 128], f32, tag="sm")
                        nc.tensor.transpose(p_t[:], x_nat[:, s, :], ident[:])
                        nc.vector.tensor_copy(xT[:, s * 128 : (s + 1) * 128], p_t[:])
                    s_exp = sexp_pool.tile([128, KC, B_MACRO], bf16, tag="s_exp")
                    cur = (x_nat, s_exp, b0)

                for k in range(KC):
                    if mi < n_mac:
                        lhsT = memT[:, k * 128 : (k + 1) * 128]
                        for h in range(N_H):
                            p_s = ps_pool.tile([128, S_W], f32, tag="ps")
                            for j in range(S_W // 512):
                                off = h * S_W + j * 512
                                nc.tensor.matmul(
                                    p_s[:, j * 512 : (j + 1) * 512],
                                    lhsT,
                                    xT[:, off : off + 512],
                                    start=True,
                                    stop=True,
                                )
                            nc.scalar.activation(
                                s_exp[:, k, h * S_W : (h + 1) * S_W], p_s[:], Exp
                            )
                    if prev is not None:
                        px_nat, ps_exp, pb0 = prev
                        s = k  # one mm2 output group per k-slot
                        p_u = pu_pool.tile([128, D], f32, tag="sm")
                        for kk in range(KC):
                            nc.tensor.matmul(
                                p_u[:],
                                ps_exp[:, kk, s * 128 : (s + 1) * 128],
                                mem_bf[:, kk, :],
                                start=(kk == 0),
                                stop=(kk == KC - 1),
                            )
                        o_t = outp.tile([128, 2 * D], f32, tag="o_t")
                        nc.vector.tensor_copy(o_t[:, :D], px_nat[:, s, :])
                        nc.vector.tensor_copy(o_t[:, D:], p_u[:])
                        nc.sync.dma_start(
                            out=o_d[pb0 + s * 128 : pb0 + (s + 1) * 128, :],
                            in_=o_t[:],
                        )
                prev = cur

    nc.compile()
    return nc


def _get_nc():
    global _built
    if _built is None:
        _built = _build()
    return _built


def run_spmd(x, memory, **spmd_kwargs):
    """Run the kernel; returns (full_output, BassKernelResults)."""
    from concourse.bass_utils import run_bass_kernel_spmd

    nc = _get_nc()
    x = np.ascontiguousarray(x, dtype=np.float32)
    memory = np.ascontiguousarray(memory, dtype=np.float32)
    in_maps = [
        {
            "x": np.ascontiguousarray(x[i * B_CORE : (i + 1) * B_CORE]),
            "memory": memory,
        }
        for i in range(N_CORES)
    ]
    res = run_bass_kernel_spmd(nc, in_maps, core_ids=list(range(N_CORES)), **spmd_kwargs)
    out = np.concatenate([res.results[i]["out"] for i in range(N_CORES)], axis=0)
    return out, res


def kernel(x, memory):
    out, _ = run_spmd(x, memory)
    return out
